# revision 10
# baseline (speedup 1.0000x reference)
"""BioXNetLayer Trainium2 kernel.

Computes, for full inputs [B=128, G=20000] and per-pathway params P=4096:
    h   = inputs @ (kernel * mapp) + bias
    h   = batchnorm(h, axis=0) * gamma + beta ; h = tanh(h)
    att = sigmoid(inputs @ att_kernel + att_bias)
    outcome = h * att
    decision = outcome @ dec_w + dec_b

Sharding: tensor-parallel over the pathway dim. Each of the 8 NeuronCores gets
a [G, 512] column shard of kernel/mapp/att_kernel (and of the per-pathway
vectors), plus a replicated transposed copy of inputs. BatchNorm stats are per
pathway over the full batch, so they are core-local. The decision head is
computed as per-core partials and summed on the host; outcome/att shards are
concatenated on the host.

Note: the pre-BN bias cancels inside batchnorm ((h+b) - mean(h+b) == h -
mean(h)), so `bias` never needs to touch the device.
"""

import sys

if "/opt/trn_rl_repo" not in sys.path:
    sys.path.insert(0, "/opt/trn_rl_repo")

from contextlib import ExitStack

import numpy as np

import concourse.bass as bass
import concourse.tile as tile
from concourse import bacc, mybir
from concourse.bass_utils import run_bass_kernel_spmd

B = 128            # batch
G = 20000          # genes
P = 4096           # pathways
NCORES = 8
PS = P // NCORES   # pathways per core
KC = 128           # genes per matmul (contraction tile)
NK = (G + KC - 1) // KC   # 157 k-subtiles
G_PAD = NK * KC           # 20096 (zero-padded genes contribute nothing)
SK = 8             # k-subtiles per DMA supertile (2 MB per weight DMA)
BN_EPS = 1e-5
F32 = mybir.dt.float32
F32R = mybir.dt.float32r  # full fp32 bits, 4x matmul throughput at N>=256
ACT = mybir.ActivationFunctionType

LAST_RESULTS = None  # BassKernelResults of the most recent device run


def build_program() -> bass.Bass:
    nc = bacc.Bacc("TRN2")

    xT = nc.dram_tensor("xT", [G_PAD, B], F32, kind="ExternalInput").ap()
    wk = nc.dram_tensor("wk", [G_PAD, PS], F32, kind="ExternalInput").ap()
    wm = nc.dram_tensor("wm", [G_PAD, PS], F32, kind="ExternalInput").ap()
    wa = nc.dram_tensor("wa", [G_PAD, PS], F32, kind="ExternalInput").ap()
    # rows: [att_bias, gamma, beta, dec_w] shards
    rows = nc.dram_tensor("rows", [4, PS], F32, kind="ExternalInput").ap()
    out_oc = nc.dram_tensor("outcome", [B, PS], F32, kind="ExternalOutput").ap()
    out_at = nc.dram_tensor("att", [B, PS], F32, kind="ExternalOutput").ap()
    out_dc = nc.dram_tensor("dec", [B, 1], F32, kind="ExternalOutput").ap()

    with ExitStack() as ctx:
        tc = ctx.enter_context(tile.TileContext(nc))
        const = ctx.enter_context(tc.tile_pool(name="const", bufs=1))
        wpool = ctx.enter_context(tc.tile_pool(name="w", bufs=2))
        xpool = ctx.enter_context(tc.tile_pool(name="x", bufs=2))
        epi = ctx.enter_context(tc.tile_pool(name="epi", bufs=1))
        mainp = ctx.enter_context(tc.tile_pool(name="mainp", bufs=1, space="PSUM"))
        tmpp = ctx.enter_context(tc.tile_pool(name="tmpp", bufs=1, space="PSUM"))

        ones_row = const.tile([1, B], F32, tag="ones_row")
        nc.vector.memset(ones_row, 1.0)
        ones_col = const.tile([B, 1], F32, tag="ones_col")
        nc.vector.memset(ones_col, 1.0)
        eps_t = const.tile([1, 1], F32, tag="eps")
        nc.vector.memset(eps_t, BN_EPS)
        # each param row in its own tile: matmul operands need base partition 0
        ab_row = const.tile([1, PS], F32, tag="ab_row")
        nc.sync.dma_start(out=ab_row, in_=rows[0:1, :])
        gm_row = const.tile([1, PS], F32, tag="gm_row")
        nc.sync.dma_start(out=gm_row, in_=rows[1:2, :])
        bt_row = const.tile([1, PS], F32, tag="bt_row")
        nc.sync.dma_start(out=bt_row, in_=rows[2:3, :])
        dw_row = const.tile([1, PS], F32, tag="dw_row")
        nc.sync.dma_start(out=dw_row, in_=rows[3:4, :])

        h_ps = mainp.tile([B, PS], F32, tag="h_ps")
        a_ps = mainp.tile([B, PS], F32, tag="a_ps")

        # Seed the attention accumulator with broadcast att_bias:
        # ones_row.T @ att_bias_row = [B, PS] of att_bias.
        nc.tensor.matmul(a_ps, lhsT=ones_row, rhs=ab_row,
                         start=True, stop=False)

        nsuper = (NK + SK - 1) // SK
        for i in range(nsuper):
            k0 = i * SK
            sk = min(SK, NK - k0)
            g0, gs = k0 * KC, sk * KC
            xt = xpool.tile([KC, SK, B], F32R, tag="xt")
            nc.sync.dma_start(
                out=xt[:, :sk, :],
                in_=xT[g0:g0 + gs, :].bitcast(F32R)
                .rearrange("(n p) b -> p n b", p=KC))
            wkt = wpool.tile([KC, SK, PS], F32R, tag="wkt")
            nc.sync.dma_start(
                out=wkt[:, :sk, :],
                in_=wk[g0:g0 + gs, :].bitcast(F32R)
                .rearrange("(n p) m -> p n m", p=KC))
            wmt = wpool.tile([KC, SK, PS], F32, tag="wmt")
            nc.sync.dma_start(
                out=wmt[:, :sk, :],
                in_=wm[g0:g0 + gs, :].rearrange("(n p) m -> p n m", p=KC))
            wat = wpool.tile([KC, SK, PS], F32R, tag="wat")
            nc.sync.dma_start(
                out=wat[:, :sk, :],
                in_=wa[g0:g0 + gs, :].bitcast(F32R)
                .rearrange("(n p) m -> p n m", p=KC))

            # mask the dense kernel: wkt *= wmt (output rounded to f32r)
            nc.vector.tensor_mul(wkt[:, :sk, :],
                                 wkt[:, :sk, :].bitcast(F32),
                                 wmt[:, :sk, :])

            for j in range(sk):
                kg = k0 + j
                last = kg == NK - 1
                nc.tensor.matmul(h_ps, lhsT=xt[:, j, :], rhs=wkt[:, j, :],
                                 start=(kg == 0), stop=last)
                nc.tensor.matmul(a_ps, lhsT=xt[:, j, :], rhs=wat[:, j, :],
                                 start=False, stop=last)

        # ---- epilogue: batchnorm + tanh + sigmoid gate + decision ----
        h_sb = epi.tile([B, PS], F32, tag="h_sb")
        nc.scalar.copy(out=h_sb, in_=h_ps)
        hsq = epi.tile([B, PS], F32, tag="hsq")
        nc.scalar.activation(out=hsq, in_=h_ps, func=ACT.Square)

        # batch sums via ones-vector matmuls (partition-axis reduction)
        sum_ps = tmpp.tile([1, PS], F32, tag="sum_ps")
        sq_ps = tmpp.tile([1, PS], F32, tag="sq_ps")
        nc.tensor.matmul(sum_ps, lhsT=ones_col, rhs=h_sb, start=True, stop=True)
        nc.tensor.matmul(sq_ps, lhsT=ones_col, rhs=hsq, start=True, stop=True)

        mean = epi.tile([1, PS], F32, tag="mean")
        nc.scalar.activation(out=mean, in_=sum_ps, func=ACT.Copy, scale=1.0 / B)
        msq = epi.tile([1, PS], F32, tag="msq")
        nc.scalar.activation(out=msq, in_=sq_ps, func=ACT.Copy, scale=1.0 / B)

        var = epi.tile([1, PS], F32, tag="var")
        nc.vector.tensor_mul(var, mean, mean)
        nc.vector.tensor_sub(var, msq, var)
        std = epi.tile([1, PS], F32, tag="std")
        nc.scalar.activation(out=std, in_=var, func=ACT.Sqrt, bias=eps_t)
        rstd = epi.tile([1, PS], F32, tag="rstd")
        nc.vector.reciprocal(out=rstd, in_=std)

        # normalized h = h * A + C with A = gamma*rstd, C = beta - mean*A
        a_row = epi.tile([1, PS], F32, tag="a_row")
        nc.vector.tensor_mul(a_row, gm_row, rstd)
        c_row = epi.tile([1, PS], F32, tag="c_row")
        nc.vector.tensor_mul(c_row, mean, a_row)
        nc.vector.tensor_sub(c_row, bt_row, c_row)

        # broadcast A, C, dec_w rows to [B, PS] via outer product with ones
        a_bc = tmpp.tile([B, PS], F32, tag="a_bc")
        nc.tensor.matmul(a_bc, lhsT=ones_row, rhs=a_row, start=True, stop=True)
        c_bc = tmpp.tile([B, PS], F32, tag="c_bc")
        nc.tensor.matmul(c_bc, lhsT=ones_row, rhs=c_row, start=True, stop=True)
        d_bc = tmpp.tile([B, PS], F32, tag="d_bc")
        nc.tensor.matmul(d_bc, lhsT=ones_row, rhs=dw_row,
                         start=True, stop=True)

        hn = epi.tile([B, PS], F32, tag="hn")
        nc.vector.tensor_mul(hn, h_sb, a_bc)
        hn2 = epi.tile([B, PS], F32, tag="hn2")
        nc.vector.tensor_add(hn2, hn, c_bc)
        th = epi.tile([B, PS], F32, tag="th")
        nc.scalar.activation(out=th, in_=hn2, func=ACT.Tanh)
        apr = epi.tile([B, PS], F32, tag="apr")
        nc.scalar.activation(out=apr, in_=a_ps, func=ACT.Sigmoid)
        oc = epi.tile([B, PS], F32, tag="oc")
        nc.vector.tensor_mul(oc, th, apr)

        nc.sync.dma_start(out=out_at, in_=apr)
        nc.sync.dma_start(out=out_oc, in_=oc)

        # decision partial: rowwise dot with dec_w shard
        dtmp = epi.tile([B, PS], F32, tag="dtmp")
        nc.vector.tensor_mul(dtmp, oc, d_bc)
        dsum = epi.tile([B, 1], F32, tag="dsum")
        nc.vector.reduce_sum(out=dsum, in_=dtmp, axis=mybir.AxisListType.X)
        nc.sync.dma_start(out=out_dc, in_=dsum)

    nc.compile()
    return nc


_PROGRAM = None


def _get_program() -> bass.Bass:
    global _PROGRAM
    if _PROGRAM is None:
        _PROGRAM = build_program()
    return _PROGRAM


def make_in_maps(inputs_np: dict) -> list[dict]:
    x = np.ascontiguousarray(np.asarray(inputs_np["inputs"], np.float32))
    ker = np.asarray(inputs_np["kernel"], np.float32)
    mp = np.asarray(inputs_np["mapp"], np.float32)
    ak = np.asarray(inputs_np["att_kernel"], np.float32)
    ab = np.asarray(inputs_np["att_bias"], np.float32)
    gm = np.asarray(inputs_np["gamma"], np.float32)
    bt = np.asarray(inputs_np["beta"], np.float32)
    dw = np.asarray(inputs_np["dec_w"], np.float32).reshape(-1)

    xT = np.zeros((G_PAD, B), np.float32)
    xT[:G] = x.T
    in_maps = []
    for c in range(NCORES):
        sl = slice(c * PS, (c + 1) * PS)
        wk_c = np.zeros((G_PAD, PS), np.float32)
        wk_c[:G] = ker[:, sl]
        wm_c = np.zeros((G_PAD, PS), np.float32)
        wm_c[:G] = mp[:, sl]
        wa_c = np.zeros((G_PAD, PS), np.float32)
        wa_c[:G] = ak[:, sl]
        rows_c = np.ascontiguousarray(np.stack([ab[sl], gm[sl], bt[sl], dw[sl]]))
        in_maps.append(
            {"xT": xT, "wk": wk_c, "wm": wm_c, "wa": wa_c, "rows": rows_c})
    return in_maps


def kernel(_run_kwargs=None, **inputs):
    global LAST_RESULTS
    in_maps = make_in_maps(inputs)
    db = np.asarray(inputs["dec_b"], np.float32)

    LAST_RESULTS = run_bass_kernel_spmd(_get_program(), in_maps,
                                        list(range(NCORES)),
                                        **(_run_kwargs or {}))
    res = LAST_RESULTS.results
    outcome = np.concatenate([res[c]["outcome"] for c in range(NCORES)], axis=1)
    att = np.concatenate([res[c]["att"] for c in range(NCORES)], axis=1)
    dec = np.sum(np.stack([res[c]["dec"] for c in range(NCORES)]), axis=0) + db
    return outcome, dec.astype(np.float32), att


# revision 14
# speedup vs baseline: 1.2108x; 1.2108x over previous
"""BioXNetLayer Trainium2 kernel.

Computes, for full inputs [B=128, G=20000] and per-pathway params P=4096:
    h   = inputs @ (kernel * mapp) + bias
    h   = batchnorm(h, axis=0) * gamma + beta ; h = tanh(h)
    att = sigmoid(inputs @ att_kernel + att_bias)
    outcome = h * att
    decision = outcome @ dec_w + dec_b

Sharding: tensor-parallel over the pathway dim. Each of the 8 NeuronCores gets
a [G, 512] column shard of kernel/mapp/att_kernel (and of the per-pathway
vectors), plus a replicated transposed copy of inputs. BatchNorm stats are per
pathway over the full batch, so they are core-local. The decision head is
computed as per-core partials and summed on the host; outcome/att shards are
concatenated on the host.

Note: the pre-BN bias cancels inside batchnorm ((h+b) - mean(h+b) == h -
mean(h)), so `bias` never needs to touch the device.
"""

import sys

if "/opt/trn_rl_repo" not in sys.path:
    sys.path.insert(0, "/opt/trn_rl_repo")

from contextlib import ExitStack

import numpy as np

import concourse.bass as bass
import concourse.tile as tile
from concourse import bacc, mybir
from concourse.bass_utils import run_bass_kernel_spmd

B = 128            # batch
G = 20000          # genes
P = 4096           # pathways
NCORES = 8
PS = P // NCORES   # pathways per core
KC = 128           # genes per matmul (contraction tile)
NK = (G + KC - 1) // KC   # 157 k-subtiles
G_PAD = NK * KC           # 20096 (zero-padded genes contribute nothing)
SK = 8             # k-subtiles per DMA supertile (2 MB per weight DMA)
BN_EPS = 1e-5
F32 = mybir.dt.float32
F32R = mybir.dt.float32r  # full fp32 bits, 4x matmul throughput at N>=256
ACT = mybir.ActivationFunctionType

LAST_RESULTS = None  # BassKernelResults of the most recent device run


def build_program() -> bass.Bass:
    nc = bacc.Bacc("TRN2")

    xT = nc.dram_tensor("xT", [G_PAD, B], F32, kind="ExternalInput").ap()
    wk = nc.dram_tensor("wk", [G_PAD, PS], F32, kind="ExternalInput").ap()
    # binary map travels as uint8: 4x less HBM traffic, exact values
    wm = nc.dram_tensor("wm", [G_PAD, PS], mybir.dt.uint8,
                        kind="ExternalInput").ap()
    wa = nc.dram_tensor("wa", [G_PAD, PS], F32, kind="ExternalInput").ap()
    # rows: [att_bias, gamma, beta, dec_w] shards
    rows = nc.dram_tensor("rows", [4, PS], F32, kind="ExternalInput").ap()
    out_oc = nc.dram_tensor("outcome", [B, PS], F32, kind="ExternalOutput").ap()
    out_at = nc.dram_tensor("att", [B, PS], F32, kind="ExternalOutput").ap()
    out_dc = nc.dram_tensor("dec", [B, 1], F32, kind="ExternalOutput").ap()

    with ExitStack() as ctx:
        tc = ctx.enter_context(tile.TileContext(nc))
        const = ctx.enter_context(tc.tile_pool(name="const", bufs=1))
        wpool = ctx.enter_context(tc.tile_pool(name="w", bufs=3))
        xpool = ctx.enter_context(tc.tile_pool(name="x", bufs=3))
        epi = ctx.enter_context(tc.tile_pool(name="epi", bufs=1))
        mainp = ctx.enter_context(tc.tile_pool(name="mainp", bufs=1, space="PSUM"))
        tmpp = ctx.enter_context(tc.tile_pool(name="tmpp", bufs=1, space="PSUM"))

        ones_row = const.tile([1, B], F32, tag="ones_row")
        nc.vector.memset(ones_row, 1.0)
        ones_col = const.tile([B, 1], F32, tag="ones_col")
        nc.vector.memset(ones_col, 1.0)
        eps_t = const.tile([1, 1], F32, tag="eps")
        nc.vector.memset(eps_t, BN_EPS)
        # each param row in its own tile: matmul operands need base partition 0
        ab_row = const.tile([1, PS], F32, tag="ab_row")
        nc.sync.dma_start(out=ab_row, in_=rows[0:1, :])
        gm_row = const.tile([1, PS], F32, tag="gm_row")
        nc.sync.dma_start(out=gm_row, in_=rows[1:2, :])
        bt_row = const.tile([1, PS], F32, tag="bt_row")
        nc.sync.dma_start(out=bt_row, in_=rows[2:3, :])
        dw_row = const.tile([1, PS], F32, tag="dw_row")
        nc.sync.dma_start(out=dw_row, in_=rows[3:4, :])

        h_ps = mainp.tile([B, PS], F32, tag="h_ps")
        a_ps = mainp.tile([B, PS], F32, tag="a_ps")

        # Seed the attention accumulator with broadcast att_bias:
        # ones_row.T @ att_bias_row = [B, PS] of att_bias.
        nc.tensor.matmul(a_ps, lhsT=ones_row, rhs=ab_row,
                         start=True, stop=False)

        nsuper = (NK + SK - 1) // SK
        for i in range(nsuper):
            k0 = i * SK
            sk = min(SK, NK - k0)
            g0, gs = k0 * KC, sk * KC
            xt = xpool.tile([KC, SK, B], F32R, tag="xt")
            nc.sync.dma_start(
                out=xt[:, :sk, :],
                in_=xT[g0:g0 + gs, :].bitcast(F32R)
                .rearrange("(n p) b -> p n b", p=KC))
            wkt = wpool.tile([KC, SK, PS], F32R, tag="wkt")
            nc.sync.dma_start(
                out=wkt[:, :sk, :],
                in_=wk[g0:g0 + gs, :].bitcast(F32R)
                .rearrange("(n p) m -> p n m", p=KC))
            wmt = wpool.tile([KC, SK, PS], mybir.dt.uint8, tag="wmt")
            nc.scalar.dma_start(
                out=wmt[:, :sk, :],
                in_=wm[g0:g0 + gs, :].rearrange("(n p) m -> p n m", p=KC))
            wat = wpool.tile([KC, SK, PS], F32R, tag="wat")
            nc.scalar.dma_start(
                out=wat[:, :sk, :],
                in_=wa[g0:g0 + gs, :].bitcast(F32R)
                .rearrange("(n p) m -> p n m", p=KC))

            # mask the dense kernel: wkt *= wmt (output rounded to f32r)
            nc.vector.tensor_mul(wkt[:, :sk, :],
                                 wkt[:, :sk, :].bitcast(F32),
                                 wmt[:, :sk, :])

            for j in range(sk):
                kg = k0 + j
                last = kg == NK - 1
                nc.tensor.matmul(h_ps, lhsT=xt[:, j, :], rhs=wkt[:, j, :],
                                 start=(kg == 0), stop=last)
                nc.tensor.matmul(a_ps, lhsT=xt[:, j, :], rhs=wat[:, j, :],
                                 start=False, stop=last)

        # ---- epilogue: batchnorm + tanh + sigmoid gate + decision ----
        h_sb = epi.tile([B, PS], F32, tag="h_sb")
        nc.scalar.copy(out=h_sb, in_=h_ps)
        hsq = epi.tile([B, PS], F32, tag="hsq")
        nc.scalar.activation(out=hsq, in_=h_ps, func=ACT.Square)

        # batch sums via ones-vector matmuls (partition-axis reduction)
        sum_ps = tmpp.tile([1, PS], F32, tag="sum_ps")
        sq_ps = tmpp.tile([1, PS], F32, tag="sq_ps")
        nc.tensor.matmul(sum_ps, lhsT=ones_col, rhs=h_sb, start=True, stop=True)
        nc.tensor.matmul(sq_ps, lhsT=ones_col, rhs=hsq, start=True, stop=True)

        mean = epi.tile([1, PS], F32, tag="mean")
        nc.scalar.activation(out=mean, in_=sum_ps, func=ACT.Copy, scale=1.0 / B)
        msq = epi.tile([1, PS], F32, tag="msq")
        nc.scalar.activation(out=msq, in_=sq_ps, func=ACT.Copy, scale=1.0 / B)

        var = epi.tile([1, PS], F32, tag="var")
        nc.vector.tensor_mul(var, mean, mean)
        nc.vector.tensor_sub(var, msq, var)
        std = epi.tile([1, PS], F32, tag="std")
        nc.scalar.activation(out=std, in_=var, func=ACT.Sqrt, bias=eps_t)
        rstd = epi.tile([1, PS], F32, tag="rstd")
        nc.vector.reciprocal(out=rstd, in_=std)

        # normalized h = h * A + C with A = gamma*rstd, C = beta - mean*A
        a_row = epi.tile([1, PS], F32, tag="a_row")
        nc.vector.tensor_mul(a_row, gm_row, rstd)
        c_row = epi.tile([1, PS], F32, tag="c_row")
        nc.vector.tensor_mul(c_row, mean, a_row)
        nc.vector.tensor_sub(c_row, bt_row, c_row)

        # broadcast A, C, dec_w rows to [B, PS] via outer product with ones
        a_bc = tmpp.tile([B, PS], F32, tag="a_bc")
        nc.tensor.matmul(a_bc, lhsT=ones_row, rhs=a_row, start=True, stop=True)
        c_bc = tmpp.tile([B, PS], F32, tag="c_bc")
        nc.tensor.matmul(c_bc, lhsT=ones_row, rhs=c_row, start=True, stop=True)
        d_bc = tmpp.tile([B, PS], F32, tag="d_bc")
        nc.tensor.matmul(d_bc, lhsT=ones_row, rhs=dw_row,
                         start=True, stop=True)

        hn = epi.tile([B, PS], F32, tag="hn")
        nc.vector.tensor_mul(hn, h_sb, a_bc)
        hn2 = epi.tile([B, PS], F32, tag="hn2")
        nc.vector.tensor_add(hn2, hn, c_bc)
        th = epi.tile([B, PS], F32, tag="th")
        nc.scalar.activation(out=th, in_=hn2, func=ACT.Tanh)
        apr = epi.tile([B, PS], F32, tag="apr")
        nc.scalar.activation(out=apr, in_=a_ps, func=ACT.Sigmoid)
        oc = epi.tile([B, PS], F32, tag="oc")
        nc.vector.tensor_mul(oc, th, apr)

        nc.sync.dma_start(out=out_at, in_=apr)
        nc.sync.dma_start(out=out_oc, in_=oc)

        # decision partial: rowwise dot with dec_w shard
        dtmp = epi.tile([B, PS], F32, tag="dtmp")
        nc.vector.tensor_mul(dtmp, oc, d_bc)
        dsum = epi.tile([B, 1], F32, tag="dsum")
        nc.vector.reduce_sum(out=dsum, in_=dtmp, axis=mybir.AxisListType.X)
        nc.sync.dma_start(out=out_dc, in_=dsum)

    nc.compile()
    return nc


_PROGRAM = None


def _get_program() -> bass.Bass:
    global _PROGRAM
    if _PROGRAM is None:
        _PROGRAM = build_program()
    return _PROGRAM


def make_in_maps(inputs_np: dict) -> list[dict]:
    x = np.ascontiguousarray(np.asarray(inputs_np["inputs"], np.float32))
    ker = np.asarray(inputs_np["kernel"], np.float32)
    mp = np.asarray(inputs_np["mapp"], np.float32)
    ak = np.asarray(inputs_np["att_kernel"], np.float32)
    ab = np.asarray(inputs_np["att_bias"], np.float32)
    gm = np.asarray(inputs_np["gamma"], np.float32)
    bt = np.asarray(inputs_np["beta"], np.float32)
    dw = np.asarray(inputs_np["dec_w"], np.float32).reshape(-1)

    xT = np.zeros((G_PAD, B), np.float32)
    xT[:G] = x.T
    in_maps = []
    for c in range(NCORES):
        sl = slice(c * PS, (c + 1) * PS)
        wk_c = np.zeros((G_PAD, PS), np.float32)
        wk_c[:G] = ker[:, sl]
        wm_c = np.zeros((G_PAD, PS), np.uint8)
        wm_c[:G] = mp[:, sl].astype(np.uint8)
        wa_c = np.zeros((G_PAD, PS), np.float32)
        wa_c[:G] = ak[:, sl]
        rows_c = np.ascontiguousarray(np.stack([ab[sl], gm[sl], bt[sl], dw[sl]]))
        in_maps.append(
            {"xT": xT, "wk": wk_c, "wm": wm_c, "wa": wa_c, "rows": rows_c})
    return in_maps


def kernel(_run_kwargs=None, **inputs):
    global LAST_RESULTS
    in_maps = make_in_maps(inputs)
    db = np.asarray(inputs["dec_b"], np.float32)

    LAST_RESULTS = run_bass_kernel_spmd(_get_program(), in_maps,
                                        list(range(NCORES)),
                                        **(_run_kwargs or {}))
    res = LAST_RESULTS.results
    outcome = np.concatenate([res[c]["outcome"] for c in range(NCORES)], axis=1)
    att = np.concatenate([res[c]["att"] for c in range(NCORES)], axis=1)
    dec = np.sum(np.stack([res[c]["dec"] for c in range(NCORES)]), axis=0) + db
    return outcome, dec.astype(np.float32), att


# revision 18
# speedup vs baseline: 1.6320x; 1.3479x over previous
"""BioXNetLayer Trainium2 kernel.

Computes, for full inputs [B=128, G=20000] and per-pathway params P=4096:
    h   = inputs @ (kernel * mapp) + bias
    h   = batchnorm(h, axis=0) * gamma + beta ; h = tanh(h)
    att = sigmoid(inputs @ att_kernel + att_bias)
    outcome = h * att
    decision = outcome @ dec_w + dec_b

Sharding: tensor-parallel over the pathway dim. Each of the 8 NeuronCores gets
a [G, 512] column shard of kernel/mapp/att_kernel (and of the per-pathway
vectors), plus a replicated transposed copy of inputs. BatchNorm stats are per
pathway over the full batch, so they are core-local. The decision head is
computed as per-core partials and summed on the host; outcome/att shards are
concatenated on the host.

Note: the pre-BN bias cancels inside batchnorm ((h+b) - mean(h+b) == h -
mean(h)), so `bias` never needs to touch the device.
"""

import sys

if "/opt/trn_rl_repo" not in sys.path:
    sys.path.insert(0, "/opt/trn_rl_repo")

from contextlib import ExitStack

import numpy as np

import concourse.bass as bass
import concourse.tile as tile
from concourse import bacc, mybir
from concourse.bass_utils import run_bass_kernel_spmd

B = 128            # batch
G = 20000          # genes
P = 4096           # pathways
NCORES = 8
PS = P // NCORES   # pathways per core
KC = 128           # genes per matmul (contraction tile)
NK = (G + KC - 1) // KC   # 157 k-subtiles
G_PAD = NK * KC           # 20096 (zero-padded genes contribute nothing)
SK = 16            # k-subtiles per DMA supertile (2 MB per weight DMA)
BN_EPS = 1e-5
F32 = mybir.dt.float32
BF16 = mybir.dt.bfloat16
ACT = mybir.ActivationFunctionType

LAST_RESULTS = None  # BassKernelResults of the most recent device run


def build_program() -> bass.Bass:
    nc = bacc.Bacc("TRN2")

    # x and the two dense [G, PS] kernels travel as bf16 (host-rounded);
    # the binary map travels as uint8. PSUM accumulation and the whole
    # epilogue stay fp32.
    xT = nc.dram_tensor("xT", [G_PAD, B], BF16, kind="ExternalInput").ap()
    wk = nc.dram_tensor("wk", [G_PAD, PS], BF16, kind="ExternalInput").ap()
    wm = nc.dram_tensor("wm", [G_PAD, PS], mybir.dt.uint8,
                        kind="ExternalInput").ap()
    wa = nc.dram_tensor("wa", [G_PAD, PS], BF16, kind="ExternalInput").ap()
    # rows: [att_bias, gamma, beta, dec_w] shards
    rows = nc.dram_tensor("rows", [4, PS], F32, kind="ExternalInput").ap()
    out_oc = nc.dram_tensor("outcome", [B, PS], F32, kind="ExternalOutput").ap()
    out_at = nc.dram_tensor("att", [B, PS], F32, kind="ExternalOutput").ap()
    out_dc = nc.dram_tensor("dec", [B, 1], F32, kind="ExternalOutput").ap()

    with ExitStack() as ctx:
        tc = ctx.enter_context(tile.TileContext(nc))
        const = ctx.enter_context(tc.tile_pool(name="const", bufs=1))
        wpool = ctx.enter_context(tc.tile_pool(name="w", bufs=3))
        xpool = ctx.enter_context(tc.tile_pool(name="x", bufs=3))
        epi = ctx.enter_context(tc.tile_pool(name="epi", bufs=1))
        mainp = ctx.enter_context(tc.tile_pool(name="mainp", bufs=1, space="PSUM"))
        tmpp = ctx.enter_context(tc.tile_pool(name="tmpp", bufs=1, space="PSUM"))

        ones_row = const.tile([1, B], F32, tag="ones_row")
        nc.vector.memset(ones_row, 1.0)
        ones_col = const.tile([B, 1], F32, tag="ones_col")
        nc.vector.memset(ones_col, 1.0)
        eps_t = const.tile([1, 1], F32, tag="eps")
        nc.vector.memset(eps_t, BN_EPS)
        # each param row in its own tile: matmul operands need base partition 0
        ab_row = const.tile([1, PS], F32, tag="ab_row")
        nc.sync.dma_start(out=ab_row, in_=rows[0:1, :])
        gm_row = const.tile([1, PS], F32, tag="gm_row")
        nc.sync.dma_start(out=gm_row, in_=rows[1:2, :])
        bt_row = const.tile([1, PS], F32, tag="bt_row")
        nc.sync.dma_start(out=bt_row, in_=rows[2:3, :])
        dw_row = const.tile([1, PS], F32, tag="dw_row")
        nc.sync.dma_start(out=dw_row, in_=rows[3:4, :])

        h_ps = mainp.tile([B, PS], F32, tag="h_ps")
        a_ps = mainp.tile([B, PS], F32, tag="a_ps")

        # Seed the attention accumulator with broadcast att_bias:
        # ones_row.T @ att_bias_row = [B, PS] of att_bias.
        nc.tensor.matmul(a_ps, lhsT=ones_row, rhs=ab_row,
                         start=True, stop=False)

        nsuper = (NK + SK - 1) // SK
        for i in range(nsuper):
            k0 = i * SK
            sk = min(SK, NK - k0)
            g0, gs = k0 * KC, sk * KC
            xt = xpool.tile([KC, SK, B], BF16, tag="xt")
            nc.sync.dma_start(
                out=xt[:, :sk, :],
                in_=xT[g0:g0 + gs, :].rearrange("(n p) b -> p n b", p=KC))
            wkt = wpool.tile([KC, SK, PS], BF16, tag="wkt")
            nc.sync.dma_start(
                out=wkt[:, :sk, :],
                in_=wk[g0:g0 + gs, :].rearrange("(n p) m -> p n m", p=KC))
            wmt = wpool.tile([KC, SK, PS], mybir.dt.uint8, tag="wmt")
            nc.scalar.dma_start(
                out=wmt[:, :sk, :],
                in_=wm[g0:g0 + gs, :].rearrange("(n p) m -> p n m", p=KC))
            wat = wpool.tile([KC, SK, PS], BF16, tag="wat")
            nc.scalar.dma_start(
                out=wat[:, :sk, :],
                in_=wa[g0:g0 + gs, :].rearrange("(n p) m -> p n m", p=KC))

            # mask the dense kernel: wkt *= wmt (x1.0 or x0.0 — exact in bf16)
            nc.vector.tensor_mul(wkt[:, :sk, :], wkt[:, :sk, :],
                                 wmt[:, :sk, :])

            for j in range(sk):
                kg = k0 + j
                last = kg == NK - 1
                nc.tensor.matmul(h_ps, lhsT=xt[:, j, :], rhs=wkt[:, j, :],
                                 start=(kg == 0), stop=last)
                nc.tensor.matmul(a_ps, lhsT=xt[:, j, :], rhs=wat[:, j, :],
                                 start=False, stop=last)

        # ---- epilogue: batchnorm + tanh + sigmoid gate + decision ----
        h_sb = epi.tile([B, PS], F32, tag="h_sb")
        nc.scalar.copy(out=h_sb, in_=h_ps)
        hsq = epi.tile([B, PS], F32, tag="hsq")
        nc.scalar.activation(out=hsq, in_=h_ps, func=ACT.Square)

        # batch sums via ones-vector matmuls (partition-axis reduction)
        sum_ps = tmpp.tile([1, PS], F32, tag="sum_ps")
        sq_ps = tmpp.tile([1, PS], F32, tag="sq_ps")
        nc.tensor.matmul(sum_ps, lhsT=ones_col, rhs=h_sb, start=True, stop=True)
        nc.tensor.matmul(sq_ps, lhsT=ones_col, rhs=hsq, start=True, stop=True)

        mean = epi.tile([1, PS], F32, tag="mean")
        nc.scalar.activation(out=mean, in_=sum_ps, func=ACT.Copy, scale=1.0 / B)
        msq = epi.tile([1, PS], F32, tag="msq")
        nc.scalar.activation(out=msq, in_=sq_ps, func=ACT.Copy, scale=1.0 / B)

        var = epi.tile([1, PS], F32, tag="var")
        nc.vector.tensor_mul(var, mean, mean)
        nc.vector.tensor_sub(var, msq, var)
        std = epi.tile([1, PS], F32, tag="std")
        nc.scalar.activation(out=std, in_=var, func=ACT.Sqrt, bias=eps_t)
        rstd = epi.tile([1, PS], F32, tag="rstd")
        nc.vector.reciprocal(out=rstd, in_=std)

        # normalized h = h * A + C with A = gamma*rstd, C = beta - mean*A
        a_row = epi.tile([1, PS], F32, tag="a_row")
        nc.vector.tensor_mul(a_row, gm_row, rstd)
        c_row = epi.tile([1, PS], F32, tag="c_row")
        nc.vector.tensor_mul(c_row, mean, a_row)
        nc.vector.tensor_sub(c_row, bt_row, c_row)

        # broadcast A, C, dec_w rows to [B, PS] via outer product with ones
        a_bc = tmpp.tile([B, PS], F32, tag="a_bc")
        nc.tensor.matmul(a_bc, lhsT=ones_row, rhs=a_row, start=True, stop=True)
        c_bc = tmpp.tile([B, PS], F32, tag="c_bc")
        nc.tensor.matmul(c_bc, lhsT=ones_row, rhs=c_row, start=True, stop=True)
        d_bc = tmpp.tile([B, PS], F32, tag="d_bc")
        nc.tensor.matmul(d_bc, lhsT=ones_row, rhs=dw_row,
                         start=True, stop=True)

        hn = epi.tile([B, PS], F32, tag="hn")
        nc.vector.tensor_mul(hn, h_sb, a_bc)
        hn2 = epi.tile([B, PS], F32, tag="hn2")
        nc.vector.tensor_add(hn2, hn, c_bc)
        th = epi.tile([B, PS], F32, tag="th")
        nc.scalar.activation(out=th, in_=hn2, func=ACT.Tanh)
        apr = epi.tile([B, PS], F32, tag="apr")
        nc.scalar.activation(out=apr, in_=a_ps, func=ACT.Sigmoid)
        oc = epi.tile([B, PS], F32, tag="oc")
        nc.vector.tensor_mul(oc, th, apr)

        nc.sync.dma_start(out=out_at, in_=apr)
        nc.sync.dma_start(out=out_oc, in_=oc)

        # decision partial: rowwise dot with dec_w shard
        dtmp = epi.tile([B, PS], F32, tag="dtmp")
        nc.vector.tensor_mul(dtmp, oc, d_bc)
        dsum = epi.tile([B, 1], F32, tag="dsum")
        nc.vector.reduce_sum(out=dsum, in_=dtmp, axis=mybir.AxisListType.X)
        nc.sync.dma_start(out=out_dc, in_=dsum)

    nc.compile()
    return nc


_PROGRAM = None


def _get_program() -> bass.Bass:
    global _PROGRAM
    if _PROGRAM is None:
        _PROGRAM = build_program()
    return _PROGRAM


def make_in_maps(inputs_np: dict) -> list[dict]:
    import ml_dtypes
    bf16 = ml_dtypes.bfloat16

    x = np.asarray(inputs_np["inputs"], np.float32)
    ker = np.asarray(inputs_np["kernel"], np.float32)
    mp = np.asarray(inputs_np["mapp"], np.float32)
    ak = np.asarray(inputs_np["att_kernel"], np.float32)
    ab = np.asarray(inputs_np["att_bias"], np.float32)
    gm = np.asarray(inputs_np["gamma"], np.float32)
    bt = np.asarray(inputs_np["beta"], np.float32)
    dw = np.asarray(inputs_np["dec_w"], np.float32).reshape(-1)

    xT = np.zeros((G_PAD, B), bf16)
    xT[:G] = x.T.astype(bf16)
    in_maps = []
    for c in range(NCORES):
        sl = slice(c * PS, (c + 1) * PS)
        wk_c = np.zeros((G_PAD, PS), bf16)
        wk_c[:G] = ker[:, sl].astype(bf16)
        wm_c = np.zeros((G_PAD, PS), np.uint8)
        wm_c[:G] = mp[:, sl].astype(np.uint8)
        wa_c = np.zeros((G_PAD, PS), bf16)
        wa_c[:G] = ak[:, sl].astype(bf16)
        rows_c = np.ascontiguousarray(np.stack([ab[sl], gm[sl], bt[sl], dw[sl]]))
        in_maps.append(
            {"xT": xT, "wk": wk_c, "wm": wm_c, "wa": wa_c, "rows": rows_c})
    return in_maps


def kernel(_run_kwargs=None, **inputs):
    global LAST_RESULTS
    in_maps = make_in_maps(inputs)
    db = np.asarray(inputs["dec_b"], np.float32)

    LAST_RESULTS = run_bass_kernel_spmd(_get_program(), in_maps,
                                        list(range(NCORES)),
                                        **(_run_kwargs or {}))
    res = LAST_RESULTS.results
    outcome = np.concatenate([res[c]["outcome"] for c in range(NCORES)], axis=1)
    att = np.concatenate([res[c]["att"] for c in range(NCORES)], axis=1)
    dec = np.sum(np.stack([res[c]["dec"] for c in range(NCORES)]), axis=0) + db
    return outcome, dec.astype(np.float32), att
